# revision 1
# baseline (speedup 1.0000x reference)
"""Trainium2 Bass kernel for the D3CG trainer-loss problem.

Computes, for full inputs:
    loss = sum((eps_theta - noise)**2)
where eps_theta is a 1x1-conv surrogate denoiser applied to
[d_t, cbct_coeffs] built from Haar DWT coefficients of x_0's two channels.

Strategy (pure data parallel over batch, 4 batches per core on 8 cores):
Everything before the square is linear in (x_0, noise) per output pixel, with
per-batch scalar coefficients. For each 64-image-row slab, [64 ct rows;
64 cb rows] sit on 128 SBUF partitions and are contracted with three
host-precomputed sparse 128x128 matrices on the tensor engine:
  - L_even against even columns, L_odd against odd columns (handles the 2x2
    Haar blocks + W mixing for both ct and cb channels in one PSUM group),
  - L_noise against a [4ch x 32row, 256] noise block (noise mixing plus the
    -noise target term).
PSUM then holds r = eps_theta - noise laid out [4*32, 256] (output-channel
blocks x rows). ScalarE Square with a per-partition bias (b + temb[t]) and
accum_out reduces each tile to per-partition partial sums; a final ones-matmul
reduces across partitions. Host sums the 8 per-core scalars.

DMA layout: the host pre-shuffles each batch's x_0 into one [128, 4096] block
(partition p = channel*64 + row-within-slab, free = (slab, col)) and noise
into [128, 2048], so each batch is ONE large SWDGE (gpsimd) DMA — SWDGE fans
descriptors across all 16 SDMA engines, where the HWDGE dynamic ring was
observed to use only 2.
"""

import sys

if "/opt/trn_rl_repo" not in sys.path:
    sys.path.insert(0, "/opt/trn_rl_repo")

import numpy as np

import concourse.bass as bass  # noqa: F401
import concourse.mybir as mybir
import concourse.tile as tile
from concourse import bacc
from concourse.bass_utils import run_bass_kernel_spmd

T = 1000
BETA_1 = 1e-4
BETA_T = 0.02

N_CORES = 8
B_TOTAL = 32
B_PER = B_TOTAL // N_CORES  # 4 batches per core
H = 512
Wd = 512
HO = H // 2   # 256 output rows
WO = Wd // 2  # 256 output cols
ROWS_PER_SLAB = 64          # image rows per slab-group (ct + cb stacked -> 128)
N_SLABS = H // ROWS_PER_SLAB  # 8
PAIRS = ROWS_PER_SLAB // 2    # 32 output rows per slab

F32 = mybir.dt.float32

# Haar 2x2 analysis kernels for [cA, cH, cV, cD] as functions of the block
# [[a, b], [c, d]] = [[x[2i,2j], x[2i,2j+1]], [x[2i+1,2j], x[2i+1,2j+1]]].
_HAAR = 0.5 * np.array(
    [
        [[1.0, 1.0], [1.0, 1.0]],    # cA
        [[1.0, 1.0], [-1.0, -1.0]],  # cH (detail axis0)
        [[1.0, -1.0], [1.0, -1.0]],  # cV (detail axis1)
        [[1.0, -1.0], [-1.0, 1.0]],  # cD
    ],
    dtype=np.float64,
)


def _schedule():
    betas = np.linspace(BETA_1, BETA_T, T, dtype=np.float64)
    return np.cumprod(1.0 - betas)


def _host_constants(W, b, temb, t):
    """Per-batch lhsT matrices + bias, float32.

    Le/Lo/Ln: [B, 128, 128] in [K, M] (lhsT) layout. bias: [128, B].
    """
    W = np.asarray(W, dtype=np.float64)
    b = np.asarray(b, dtype=np.float64)
    temb = np.asarray(temb, dtype=np.float64)
    t = np.asarray(t).astype(np.int64)

    alphas_bar = _schedule()
    s_ab = np.sqrt(alphas_bar[t])          # [B]
    s_omab = np.sqrt(1.0 - alphas_bar[t])  # [B]

    B = t.shape[0]
    Le = np.zeros((B, 128, 128), dtype=np.float64)
    Lo = np.zeros((B, 128, 128), dtype=np.float64)
    Ln = np.zeros((B, 128, 128), dtype=np.float64)
    bias = np.zeros((128, B), dtype=np.float64)

    for bi in range(B):
        # eps[o] = s_ab * sum_k W[o,k] haar_k(ct)
        #        + sum_k (W[o,4+k] - s_ab W[o,k]) haar_k(cb)
        #        + s_omab * sum_c W[o,c] n_c + b[o] + temb[t,o]
        # r = eps - noise -> noise coeff C[o,c] = s_omab*W[o,c] - delta_oc
        KA = np.einsum("ok,krc->orc", W[:, 0:4], _HAAR) * s_ab[bi]       # [4,2,2]
        KB = np.einsum("ok,krc->orc", W[:, 4:8] - s_ab[bi] * W[:, 0:4], _HAAR)
        C = s_omab[bi] * W[:, 0:4] - np.eye(4)                            # [4,4]

        for o in range(4):
            for i in range(PAIRS):
                m = o * PAIRS + i
                for r in range(2):
                    # ct rows occupy slab partitions 0..63, cb rows 64..127
                    Le[bi, 2 * i + r, m] = KA[o, r, 0]
                    Lo[bi, 2 * i + r, m] = KA[o, r, 1]
                    Le[bi, 64 + 2 * i + r, m] = KB[o, r, 0]
                    Lo[bi, 64 + 2 * i + r, m] = KB[o, r, 1]
                for c in range(4):
                    Ln[bi, c * PAIRS + i, m] = C[o, c]
            bias[o * PAIRS : (o + 1) * PAIRS, bi] = b[o] + temb[t[bi], o]

    return (
        Le.astype(np.float32),
        Lo.astype(np.float32),
        Ln.astype(np.float32),
        bias.astype(np.float32),
    )


def _shuffle_x0(x0_shard):
    """[B,2,512,512] -> [B, 128, N_SLABS*Wd]; partition p = c*64 + (row%64),
    free = (slab, col)."""
    B = x0_shard.shape[0]
    v = x0_shard.reshape(B, 2, N_SLABS, ROWS_PER_SLAB, Wd)
    return np.ascontiguousarray(
        v.transpose(0, 1, 3, 2, 4).reshape(B, 128, N_SLABS * Wd)
    )


def _shuffle_nz(nz_shard):
    """[B,4,256,256] -> [B, 128, N_SLABS*WO]; partition p = c*32 + (row%32)."""
    B = nz_shard.shape[0]
    v = nz_shard.reshape(B, 4, N_SLABS, PAIRS, WO)
    return np.ascontiguousarray(
        v.transpose(0, 1, 3, 2, 4).reshape(B, 128, N_SLABS * WO)
    )


def build_nc(debug=False):
    """Build the per-core Bass program (same program on all 8 cores)."""
    nc = bacc.Bacc("TRN2", target_bir_lowering=False, debug=debug)

    x0_d = nc.declare_dram_parameter(
        "x0", [B_PER, 128, N_SLABS * Wd], F32, isOutput=False
    )
    nz_d = nc.declare_dram_parameter(
        "nz", [B_PER, 128, N_SLABS * WO], F32, isOutput=False
    )
    # lhsT weights, host-pretransposed to [K=128, b, M=128]
    le_d = nc.declare_dram_parameter("Le", [128, B_PER, 128], F32, isOutput=False)
    lo_d = nc.declare_dram_parameter("Lo", [128, B_PER, 128], F32, isOutput=False)
    ln_d = nc.declare_dram_parameter("Ln", [128, B_PER, 128], F32, isOutput=False)
    bias_d = nc.declare_dram_parameter("bias", [128, B_PER], F32, isOutput=False)
    out_d = nc.declare_dram_parameter("out", [1, 1], F32, isOutput=True)

    with tile.TileContext(nc) as tc:
        with (
            tc.tile_pool(name="consts", bufs=1) as consts,
            tc.tile_pool(name="slab", bufs=2) as slab_pool,
            tc.tile_pool(name="nzp", bufs=2) as nz_pool,
            tc.tile_pool(name="sq", bufs=4) as sq_pool,
            tc.tile_pool(name="psum", bufs=6, space="PSUM") as psum_pool,
            tc.tile_pool(name="psum_fin", bufs=1, space="PSUM") as psum_fin,
        ):
            le_t = consts.tile([128, B_PER, 128], F32, tag="le_t")
            lo_t = consts.tile([128, B_PER, 128], F32, tag="lo_t")
            ln_t = consts.tile([128, B_PER, 128], F32, tag="ln_t")
            bias_t = consts.tile([128, B_PER], F32, tag="bias_t")
            partials = consts.tile([128, B_PER * N_SLABS], F32, tag="partials")

            nc.sync.dma_start(le_t[:], le_d[:])
            nc.sync.dma_start(lo_t[:], lo_d[:])
            nc.sync.dma_start(ln_t[:], ln_d[:])
            nc.sync.dma_start(bias_t[:], bias_d[:])

            for b in range(B_PER):
                # one big SWDGE DMA per batch for x0 and for noise
                xt = slab_pool.tile([128, N_SLABS, WO, 2], F32)
                nc.gpsimd.dma_start(xt[:], x0_d[b])
                nzt = nz_pool.tile([128, N_SLABS, WO], F32)
                nc.gpsimd.dma_start(nzt[:], nz_d[b])

                for g in range(N_SLABS):
                    ps = psum_pool.tile([128, WO], F32)
                    nc.tensor.matmul(
                        ps[:], le_t[:, b, :], xt[:, g, :, 0], start=True, stop=False
                    )
                    nc.tensor.matmul(
                        ps[:], lo_t[:, b, :], xt[:, g, :, 1], start=False, stop=False
                    )
                    nc.tensor.matmul(
                        ps[:], ln_t[:, b, :], nzt[:, g, :], start=False, stop=True
                    )

                    sq = sq_pool.tile([128, WO], F32)
                    col = b * N_SLABS + g
                    nc.scalar.activation(
                        sq[:],
                        ps[:],
                        mybir.ActivationFunctionType.Square,
                        bias=bias_t[:, b : b + 1],
                        scale=1.0,
                        accum_out=partials[:, col : col + 1],
                    )

            # reduce [128, 32] partials -> [128, 1] -> scalar via ones-matmul
            red = consts.tile([128, 1], F32, tag="red")
            nc.vector.tensor_reduce(
                red[:], partials[:], axis=mybir.AxisListType.X, op=mybir.AluOpType.add
            )
            ones = consts.tile([128, 1], F32, tag="ones")
            nc.gpsimd.memset(ones[:], 1.0)
            fin = psum_fin.tile([1, 1], F32, tag="fin")
            nc.tensor.matmul(fin[:], red[:], ones[:], start=True, stop=True)
            out_sb = consts.tile([1, 1], F32, tag="out_sb")
            nc.vector.tensor_copy(out_sb[:], fin[:])
            nc.sync.dma_start(out_d[:], out_sb[:])

    nc.compile()
    return nc


_NC_CACHE = None


def _get_nc():
    global _NC_CACHE
    if _NC_CACHE is None:
        _NC_CACHE = build_nc()
    return _NC_CACHE


def make_in_maps(x_0, noise, W, b, temb, t):
    x_0 = np.asarray(x_0, dtype=np.float32)
    noise = np.asarray(noise, dtype=np.float32)
    Le, Lo, Ln, bias = _host_constants(W, b, temb, t)

    in_maps = []
    for c in range(N_CORES):
        s = slice(c * B_PER, (c + 1) * B_PER)
        in_maps.append(
            {
                "x0": _shuffle_x0(x_0[s]),
                "nz": _shuffle_nz(noise[s]),
                "Le": np.ascontiguousarray(Le[s].transpose(1, 0, 2)),
                "Lo": np.ascontiguousarray(Lo[s].transpose(1, 0, 2)),
                "Ln": np.ascontiguousarray(Ln[s].transpose(1, 0, 2)),
                "bias": np.ascontiguousarray(bias[:, s]),
            }
        )
    return in_maps


def kernel(x_0, noise, W, b, temb, t, **_ignored):
    nc = _get_nc()
    in_maps = make_in_maps(x_0, noise, W, b, temb, t)
    res = run_bass_kernel_spmd(nc, in_maps, list(range(N_CORES)))
    total = 0.0
    for c in range(N_CORES):
        total += float(res.results[c]["out"][0, 0])
    return np.float32(total)



# revision 3
# speedup vs baseline: 1.8120x; 1.8120x over previous
"""Trainium2 Bass kernel for the D3CG trainer-loss problem.

Computes, for full inputs:
    loss = sum((eps_theta - noise)**2)
where eps_theta is a 1x1-conv surrogate denoiser applied to
[d_t, cbct_coeffs] built from Haar DWT coefficients of x_0's two channels.

Strategy (pure data parallel over batch, 4 batches per core on 8 cores):
Everything before the square is linear in (x_0, noise) per output pixel, with
per-batch scalar coefficients, so each 64-image-row slab reduces to matmuls
against small host-precomputed coefficient matrices:

  r = Le^T @ x_even + Lo^T @ x_odd + (-I + NzB)^T @ noise  (+ bias, per batch)

laid out [4 outch x 32 rowpairs, 256 cols] in PSUM. All data is streamed in
fp8e4m3 (host casts; the final loss tolerates it: simulated rel err ~2.6e-3
vs the 2e-2 gate) and both matmuls per slab run in DoubleRow perf mode
(0.5 PE cycles/row):
  - x-matmul: k-tile0 = Le vs even columns, k-tile1 = Lo vs odd columns.
  - noise-matmul: k-tile0 = exact -I (representable in fp8), k-tile1 =
    s_omab*W mixing pattern (small coefficients, fp8-safe); the rhs broadcasts
    the same noise tile to both k-tiles with a stride-0 AP.
ScalarE Square with per-partition bias (b + temb[t]) and accum_out reduces
each half-batch PSUM tile [128, 1024] to per-partition partial sums; the
[128, 8] partials are DMAed out and summed on host together with the
cross-core reduction.

DMA: one fp8 blob per batch [128, 6656] (coefs | x/noise first-half | x/noise
second-half) issued as two SWDGE DMAs so compute on slabs 0-3 starts while
slabs 4-7 are still in flight. 16 SDMA engines stream ~285 GB/s; total HBM
traffic is ~3.3 MiB/core (vs 12.8 MiB for the fp32 version).
"""

import sys

if "/opt/trn_rl_repo" not in sys.path:
    sys.path.insert(0, "/opt/trn_rl_repo")

import numpy as np
import ml_dtypes

import concourse.bass as bass  # noqa: F401
import concourse.mybir as mybir
import concourse.tile as tile
from concourse import bacc
from concourse.bass_utils import run_bass_kernel_spmd

T = 1000
BETA_1 = 1e-4
BETA_T = 0.02

N_CORES = 8
B_TOTAL = 32
B_PER = B_TOTAL // N_CORES  # 4 batches per core
H = 512
Wd = 512
N_SLABS = 8                 # 64-image-row slabs per image
PAIRS = 32                  # output rows per slab
WO = 256                    # output cols

F32 = mybir.dt.float32
FP8 = mybir.dt.float8e4
NPFP8 = ml_dtypes.float8_e4m3

# per-partition byte offsets inside the per-batch fp8 blob [128, 6656]
OFF_COEF = 0        # LeLo [2,128] + NzMat [2,128] = 512 B
OFF_XA = 512        # x slabs 0-3: [4, 2, 256] = 2048 B
OFF_NZA = 2560      # nz slabs 0-3: [4, 256] = 1024 B
OFF_XB = 3584       # x slabs 4-7
OFF_NZB = 5632      # nz slabs 4-7
BLOB = 6656

# Haar 2x2 analysis kernels for [cA, cH, cV, cD] as functions of the block
# [[a, b], [c, d]] = [[x[2i,2j], x[2i,2j+1]], [x[2i+1,2j], x[2i+1,2j+1]]].
_HAAR = 0.5 * np.array(
    [
        [[1.0, 1.0], [1.0, 1.0]],    # cA
        [[1.0, 1.0], [-1.0, -1.0]],  # cH (detail axis0)
        [[1.0, -1.0], [1.0, -1.0]],  # cV (detail axis1)
        [[1.0, -1.0], [-1.0, 1.0]],  # cD
    ],
    dtype=np.float64,
)


def _schedule():
    betas = np.linspace(BETA_1, BETA_T, T, dtype=np.float64)
    return np.cumprod(1.0 - betas)


def _host_constants(W, b, temb, t):
    """Coefficient blob [B, 128, 512] fp8 + bias [128, B] fp32.

    Per batch, per partition p (k-row): bytes [0:128) Le, [128:256) Lo,
    [256:384) -I, [384:512) NzB = s_omab*W pattern; columns are the matmul
    M index (4 outch x 32 rowpairs).
    """
    W = np.asarray(W, dtype=np.float64)
    b = np.asarray(b, dtype=np.float64)
    temb = np.asarray(temb, dtype=np.float64)
    t = np.asarray(t).astype(np.int64)

    alphas_bar = _schedule()
    s_ab = np.sqrt(alphas_bar[t])
    s_omab = np.sqrt(1.0 - alphas_bar[t])

    B = t.shape[0]
    coef = np.zeros((B, 128, 4, 128), dtype=np.float64)
    bias = np.zeros((128, B), dtype=np.float64)

    i_idx = np.arange(PAIRS)
    for bi in range(B):
        KA = np.einsum("ok,krc->orc", W[:, 0:4], _HAAR) * s_ab[bi]
        KB = np.einsum("ok,krc->orc", W[:, 4:8] - s_ab[bi] * W[:, 0:4], _HAAR)
        for o in range(4):
            m = o * PAIRS + i_idx
            for r in range(2):
                coef[bi, 2 * i_idx + r, 0, m] = KA[o, r, 0]
                coef[bi, 2 * i_idx + r, 1, m] = KA[o, r, 1]
                coef[bi, 64 + 2 * i_idx + r, 0, m] = KB[o, r, 0]
                coef[bi, 64 + 2 * i_idx + r, 1, m] = KB[o, r, 1]
            for c in range(4):
                coef[bi, c * PAIRS + i_idx, 2, m] = -1.0 if o == c else 0.0
                coef[bi, c * PAIRS + i_idx, 3, m] = s_omab[bi] * W[o, c]
            bias[m, bi] = b[o] + temb[t[bi], o]

    return (
        coef.astype(NPFP8).reshape(B, 128, 512),
        bias.astype(np.float32),
    )


def _shuffle_x0(x0_shard):
    """[B,2,512,512] fp32 -> [B, 128, 8, 512] fp8; partition p = ch*64 +
    (row%64); free = (slab, parity, colpair)."""
    B = x0_shard.shape[0]
    v = x0_shard.reshape(B, 2, N_SLABS, 64, WO, 2)
    v = v.transpose(0, 1, 3, 2, 5, 4)  # (B, ch, row64, slab, parity, colpair)
    return np.ascontiguousarray(v.reshape(B, 128, N_SLABS, 512)).astype(NPFP8)


def _shuffle_nz(nz_shard):
    """[B,4,256,256] fp32 -> [B, 128, 8, 256] fp8; partition p = ch*32 + pair%32."""
    B = nz_shard.shape[0]
    v = nz_shard.reshape(B, 4, N_SLABS, PAIRS, WO)
    v = v.transpose(0, 1, 3, 2, 4)
    return np.ascontiguousarray(v.reshape(B, 128, N_SLABS, WO)).astype(NPFP8)


def build_nc(debug=False):
    """Build the per-core Bass program (same program on all 8 cores)."""
    nc = bacc.Bacc("TRN2", target_bir_lowering=False, debug=debug)

    data_d = nc.declare_dram_parameter("data", [B_PER, 128, BLOB], FP8, isOutput=False)
    bias_d = nc.declare_dram_parameter("bias", [128, B_PER], F32, isOutput=False)
    out_d = nc.declare_dram_parameter("out", [128, 2 * B_PER], F32, isOutput=True)

    DR = mybir.MatmulPerfMode.DoubleRow

    with tile.TileContext(nc) as tc:
        with (
            tc.tile_pool(name="consts", bufs=1) as consts,
            tc.tile_pool(name="blob", bufs=2) as blob_pool,
            tc.tile_pool(name="sq", bufs=2) as sq_pool,
            tc.tile_pool(name="psum", bufs=2, space="PSUM") as psum_pool,
        ):
            bias_t = consts.tile([128, B_PER], F32, tag="bias_t")
            partials = consts.tile([128, 2 * B_PER], F32, tag="partials")
            nc.gpsimd.dma_start(bias_t[:], bias_d[:])

            for b in range(B_PER):
                dt = blob_pool.tile([128, BLOB], FP8)
                # two DMAs into disjoint halves so slabs 0-3 compute while
                # slabs 4-7 are still in flight
                nc.gpsimd.dma_start(dt[:, 0:OFF_XB], data_d[b, :, 0:OFF_XB])
                nc.gpsimd.dma_start(dt[:, OFF_XB:BLOB], data_d[b, :, OFF_XB:BLOB])

                lelo = dt[:, OFF_COEF : OFF_COEF + 256].rearrange(
                    "p (k m) -> p k m", k=2
                )
                nzmat = dt[:, OFF_COEF + 256 : OFF_COEF + 512].rearrange(
                    "p (k m) -> p k m", k=2
                )

                ps = psum_pool.tile([128, 2048], F32)
                for g in range(N_SLABS):
                    if g < 4:
                        xoff = OFF_XA + g * 512
                        noff = OFF_NZA + g * 256
                    else:
                        xoff = OFF_XB + (g - 4) * 512
                        noff = OFF_NZB + (g - 4) * 256
                    xg = dt[:, xoff : xoff + 512].rearrange("p (k n) -> p k n", k=2)
                    nzg = (
                        dt[:, noff : noff + 256]
                        .unsqueeze(1)
                        .broadcast_to([128, 2, 256])
                    )
                    seg = ps[:, g * 256 : (g + 1) * 256]
                    # one accumulation group per 2 KiB PSUM bank (= 2 slabs):
                    # start_tensor_calc zeroes the whole bank, so the group
                    # must cover both 256-col halves before stopping.
                    nc.tensor.matmul(seg, lelo, xg, start=(g % 2 == 0),
                                     stop=False, perf_mode=DR,
                                     skip_group_check=True)
                    nc.tensor.matmul(seg, nzmat, nzg, start=False,
                                     stop=(g % 2 == 1), perf_mode=DR,
                                     skip_group_check=True)

                    if g % 4 == 3:
                        # square + per-partition accumulate for this half-batch
                        half = (g - 3) * 256
                        sq = sq_pool.tile([128, 1024], F32)
                        col = 2 * b + g // 4
                        nc.scalar.activation(
                            sq[:],
                            ps[:, half : half + 1024],
                            mybir.ActivationFunctionType.Square,
                            bias=bias_t[:, b : b + 1],
                            scale=1.0,
                            accum_out=partials[:, col : col + 1],
                        )

            nc.sync.dma_start(out_d[:], partials[:])

    nc.compile()
    return nc


_NC_CACHE = None


def _get_nc():
    global _NC_CACHE
    if _NC_CACHE is None:
        _NC_CACHE = build_nc()
    return _NC_CACHE


def make_in_maps(x_0, noise, W, b, temb, t):
    x_0 = np.asarray(x_0, dtype=np.float32)
    noise = np.asarray(noise, dtype=np.float32)
    coef, bias = _host_constants(W, b, temb, t)
    xs = _shuffle_x0(x_0)    # [B, 128, 8, 512] fp8
    ns = _shuffle_nz(noise)  # [B, 128, 8, 256] fp8

    in_maps = []
    for c in range(N_CORES):
        s = slice(c * B_PER, (c + 1) * B_PER)
        xc, nc_, cc = xs[s], ns[s], coef[s]
        blob = np.empty((B_PER, 128, BLOB), dtype=NPFP8)
        blob[:, :, OFF_COEF:OFF_XA] = cc
        blob[:, :, OFF_XA:OFF_NZA] = xc[:, :, :4].reshape(B_PER, 128, 2048)
        blob[:, :, OFF_NZA:OFF_XB] = nc_[:, :, :4].reshape(B_PER, 128, 1024)
        blob[:, :, OFF_XB:OFF_NZB] = xc[:, :, 4:].reshape(B_PER, 128, 2048)
        blob[:, :, OFF_NZB:BLOB] = nc_[:, :, 4:].reshape(B_PER, 128, 1024)
        in_maps.append(
            {
                "data": blob,
                "bias": np.ascontiguousarray(bias[:, s]),
            }
        )
    return in_maps


def kernel(x_0, noise, W, b, temb, t, **_ignored):
    nc = _get_nc()
    in_maps = make_in_maps(x_0, noise, W, b, temb, t)
    res = run_bass_kernel_spmd(nc, in_maps, list(range(N_CORES)))
    total = 0.0
    for c in range(N_CORES):
        total += float(res.results[c]["out"].astype(np.float64).sum())
    return np.float32(total)


# revision 15
# speedup vs baseline: 2.5045x; 1.3822x over previous
"""Trainium2 Bass kernel for the D3CG trainer-loss problem.

Computes, for full inputs:
    loss = sum((eps_theta - noise)**2)
where eps_theta is a 1x1-conv surrogate denoiser applied to
[d_t, cbct_coeffs] built from Haar DWT coefficients of x_0's two channels.

Strategy (pure data parallel over batch, 4 batches per core on 8 cores):
Everything before the square is linear in (x_0, noise) per output pixel, with
per-batch scalar coefficients, so each 64-image-row slab reduces to matmuls
against small host-precomputed coefficient matrices:

  r = Le^T @ x_even + Lo^T @ x_odd + (-I + NzB)^T @ noise  (+ bias, per batch)

laid out [4 outch x 32 rowpairs, 256 cols] in PSUM. All data is streamed in
fp8e4m3 (host casts; the final loss tolerates it: simulated rel err ~2.6e-3
vs the 2e-2 gate) and both matmuls per slab run in DoubleRow perf mode
(0.5 PE cycles/row):
  - x-matmul: k-tile0 = Le vs even columns, k-tile1 = Lo vs odd columns.
  - noise-matmul: k-tile0 = exact -I (representable in fp8), k-tile1 =
    s_omab*W mixing pattern (small coefficients, fp8-safe); the rhs broadcasts
    the same noise tile to both k-tiles with a stride-0 AP.
One ScalarE Square per batch with per-partition bias (b + temb[t], carried
as 4 fp32 bytes inside the batch blob) and accum_out reduces the [128, 2048]
PSUM tile to per-partition partial sums; the [128, 4] partials are DMAed out
and summed on host together with the cross-core reduction.

DMA: one uint8 blob per batch [128, 6660] (coefs | x/nz slabs 0-3 | x/nz
slabs 4-7 | bias), bitcast to fp8 on device, issued as two transfers on the
two HWDGE dynamic rings (sync + scalar engines, even/odd batches) so all
triggers fire right after the NEFF prologue and batch 0 computes while later
batches stream. Total HBM traffic ~3.3 MiB/core (vs 12.8 MiB fp32).
"""

import sys

if "/opt/trn_rl_repo" not in sys.path:
    sys.path.insert(0, "/opt/trn_rl_repo")

import numpy as np
import ml_dtypes

import concourse.bass as bass  # noqa: F401
import concourse.mybir as mybir
import concourse.tile as tile
from concourse import bacc
from concourse.bass_utils import run_bass_kernel_spmd

T = 1000
BETA_1 = 1e-4
BETA_T = 0.02

N_CORES = 8
B_TOTAL = 32
B_PER = B_TOTAL // N_CORES  # 4 batches per core
H = 512
Wd = 512
N_SLABS = 8                 # 64-image-row slabs per image
PAIRS = 32                  # output rows per slab
WO = 256                    # output cols

F32 = mybir.dt.float32
FP8 = mybir.dt.float8e4
NPFP8 = ml_dtypes.float8_e4m3

# per-partition byte offsets inside the per-batch fp8 blob [128, 6656]
OFF_COEF = 0        # LeLo [2,128] + NzMat [2,128] = 512 B
OFF_XA = 512        # x slab-pairs 0-1: [2, 2 parity, 2 slab, 256] = 2048 B
OFF_NZA = 2560      # nz slabs 0-3: [4, 256] = 1024 B
OFF_XB = 3584       # x slab-pairs 2-3
OFF_NZB = 5632      # nz slabs 4-7
OFF_BIAS = 6656     # this batch's bias column [128,1] fp32 = 4 B
BLOB = 6660

# Haar 2x2 analysis kernels for [cA, cH, cV, cD] as functions of the block
# [[a, b], [c, d]] = [[x[2i,2j], x[2i,2j+1]], [x[2i+1,2j], x[2i+1,2j+1]]].
_HAAR = 0.5 * np.array(
    [
        [[1.0, 1.0], [1.0, 1.0]],    # cA
        [[1.0, 1.0], [-1.0, -1.0]],  # cH (detail axis0)
        [[1.0, -1.0], [1.0, -1.0]],  # cV (detail axis1)
        [[1.0, -1.0], [-1.0, 1.0]],  # cD
    ],
    dtype=np.float64,
)


def _schedule():
    betas = np.linspace(BETA_1, BETA_T, T, dtype=np.float64)
    return np.cumprod(1.0 - betas)


def _host_constants(W, b, temb, t):
    """Coefficient blob [B, 128, 512] fp8 + bias [128, B] fp32.

    Per batch, per partition p (k-row): bytes [0:128) Le, [128:256) Lo,
    [256:384) -I, [384:512) NzB = s_omab*W pattern; columns are the matmul
    M index (4 outch x 32 rowpairs).
    """
    W = np.asarray(W, dtype=np.float64)
    b = np.asarray(b, dtype=np.float64)
    temb = np.asarray(temb, dtype=np.float64)
    t = np.asarray(t).astype(np.int64)

    alphas_bar = _schedule()
    s_ab = np.sqrt(alphas_bar[t])
    s_omab = np.sqrt(1.0 - alphas_bar[t])

    B = t.shape[0]
    coef = np.zeros((B, 128, 4, 128), dtype=np.float64)
    bias = np.zeros((128, B), dtype=np.float64)

    i_idx = np.arange(PAIRS)
    for bi in range(B):
        KA = np.einsum("ok,krc->orc", W[:, 0:4], _HAAR) * s_ab[bi]
        KB = np.einsum("ok,krc->orc", W[:, 4:8] - s_ab[bi] * W[:, 0:4], _HAAR)
        for o in range(4):
            m = o * PAIRS + i_idx
            for r in range(2):
                coef[bi, 2 * i_idx + r, 0, m] = KA[o, r, 0]
                coef[bi, 2 * i_idx + r, 1, m] = KA[o, r, 1]
                coef[bi, 64 + 2 * i_idx + r, 0, m] = KB[o, r, 0]
                coef[bi, 64 + 2 * i_idx + r, 1, m] = KB[o, r, 1]
            for c in range(4):
                coef[bi, c * PAIRS + i_idx, 2, m] = -1.0 if o == c else 0.0
                coef[bi, c * PAIRS + i_idx, 3, m] = s_omab[bi] * W[o, c]
            bias[m, bi] = b[o] + temb[t[bi], o]

    return (
        coef.astype(NPFP8).reshape(B, 128, 512),
        bias.astype(np.float32),
    )


def _shuffle_x0(x0_shard):
    """[B,2,512,512] fp32 -> [B, 128, 4, 2, 2, 256] fp8; partition p = ch*64 +
    (row%64); free = (slabpair, parity, slab01, colpair) so one DoubleRow
    matmul covers a 512-col PSUM bank (2 slabs)."""
    B = x0_shard.shape[0]
    v = x0_shard.reshape(B, 2, 4, 2, 64, WO, 2)
    # (B, ch, pair, slab01, row64, colpair, parity)
    v = v.transpose(0, 1, 4, 2, 6, 3, 5)
    # -> (B, ch, row64, pair, parity, slab01, colpair)
    return np.ascontiguousarray(v.reshape(B, 128, 4, 1024)).astype(NPFP8)


def _shuffle_nz(nz_shard):
    """[B,4,256,256] fp32 -> [B, 128, 8, 256] fp8; partition p = ch*32 + pair%32."""
    B = nz_shard.shape[0]
    v = nz_shard.reshape(B, 4, N_SLABS, PAIRS, WO)
    v = v.transpose(0, 1, 3, 2, 4)
    return np.ascontiguousarray(v.reshape(B, 128, N_SLABS, WO)).astype(NPFP8)


def build_nc(debug=False):
    """Build the per-core Bass program (same program on all 8 cores)."""
    nc = bacc.Bacc("TRN2", target_bir_lowering=False, debug=debug)

    # uint8 on the wire: fp8 arrays take a flaky path through the PJRT/axon
    # upload, a same-byte uint8 view does not; APs bitcast to fp8 on device
    U8 = mybir.dt.uint8
    data_d = nc.declare_dram_parameter("data", [B_PER, 128, BLOB], U8, isOutput=False)
    out_d = nc.declare_dram_parameter("out", [128, B_PER], F32, isOutput=True)

    DR = mybir.MatmulPerfMode.DoubleRow

    with tile.TileContext(nc) as tc:
        with (
            tc.tile_pool(name="consts", bufs=1) as consts,
            tc.tile_pool(name="blob", bufs=4) as blob_pool,
            tc.tile_pool(name="sq", bufs=2) as sq_pool,
            tc.tile_pool(name="psum", bufs=2, space="PSUM") as psum_pool,
        ):
            partials = consts.tile([128, B_PER], F32, tag="partials")
            warm = consts.tile([128, 1], F32, tag="warm")
            # warm up the Square activation table during the DMA ramp so the
            # ~1.3us ACT_TABLE_LOAD is off the critical path
            nc.gpsimd.memset(warm[:], 0.0)
            nc.scalar.activation(
                warm[:], warm[:], mybir.ActivationFunctionType.Square
            )

            # all four blobs resident (bufs=4): every DMA trigger fires at
            # t~0 with no WAR hazards. Two HWDGE rings (sync/scalar), each
            # blob in two halves so batch 0's matmuls start ~2us earlier.
            tiles = []
            for b in range(B_PER):
                dt = blob_pool.tile([128, BLOB], U8)
                eng = nc.sync if b % 2 == 0 else nc.scalar
                eng.dma_start(dt[:, 0:OFF_XB], data_d[b, :, 0:OFF_XB])
                eng.dma_start(dt[:, OFF_XB:BLOB], data_d[b, :, OFF_XB:BLOB])
                tiles.append(dt)

            for b in range(B_PER):
                dt = tiles[b]
                lelo = (
                    dt[:, OFF_COEF : OFF_COEF + 256]
                    .bitcast(FP8)
                    .rearrange("p (k m) -> p k m", k=2)
                )
                nzmat = (
                    dt[:, OFF_COEF + 256 : OFF_COEF + 512]
                    .bitcast(FP8)
                    .rearrange("p (k m) -> p k m", k=2)
                )

                ps = psum_pool.tile([128, 2048], F32)
                for p in range(4):  # slab-pairs; one 512-col PSUM bank each
                    if p < 2:
                        xoff = OFF_XA + p * 1024
                        noff = OFF_NZA + p * 512
                    else:
                        xoff = OFF_XB + (p - 2) * 1024
                        noff = OFF_NZB + (p - 2) * 512
                    xg = (
                        dt[:, xoff : xoff + 1024]
                        .bitcast(FP8)
                        .rearrange("p (k n) -> p k n", k=2)
                    )
                    nzg = (
                        dt[:, noff : noff + 512]
                        .bitcast(FP8)
                        .unsqueeze(1)
                        .broadcast_to([128, 2, 512])
                    )
                    seg = ps[:, p * 512 : (p + 1) * 512]
                    nc.tensor.matmul(seg, lelo, xg, start=True, stop=False,
                                     perf_mode=DR)
                    nc.tensor.matmul(seg, nzmat, nzg, start=False, stop=True,
                                     perf_mode=DR)

                # one Square per batch over the whole PSUM tile: fewer
                # Scalar instructions -> ~2.2us/batch cadence instead of 2.8
                sq = sq_pool.tile([128, 2048], F32)
                bias_ap = dt[:, OFF_BIAS : OFF_BIAS + 4].bitcast(F32)
                nc.scalar.activation(
                    sq[:],
                    ps[:, 0:2048],
                    mybir.ActivationFunctionType.Square,
                    bias=bias_ap,
                    scale=1.0,
                    accum_out=partials[:, b : b + 1],
                )

            # copy-then-DMA: the copy reads every partials column, pinning the
            # out DMA behind all eight accumulator dumps
            out_sb = consts.tile([128, B_PER], F32, tag="out_sb")
            nc.vector.tensor_copy(out_sb[:], partials[:])
            nc.sync.dma_start(out_d[:], out_sb[:])

    nc.compile()
    return nc


_NC_CACHE = None


def _get_nc():
    global _NC_CACHE
    if _NC_CACHE is None:
        _NC_CACHE = build_nc()
    return _NC_CACHE


def make_in_maps(x_0, noise, W, b, temb, t):
    x_0 = np.asarray(x_0, dtype=np.float32)
    noise = np.asarray(noise, dtype=np.float32)
    coef, bias = _host_constants(W, b, temb, t)
    xs = _shuffle_x0(x_0)    # [B, 128, 8, 512] fp8
    ns = _shuffle_nz(noise)  # [B, 128, 8, 256] fp8

    in_maps = []
    for c in range(N_CORES):
        s = slice(c * B_PER, (c + 1) * B_PER)
        xc, nc_, cc = xs[s], ns[s], coef[s]
        blob = np.empty((B_PER, 128, BLOB), dtype=np.uint8)
        bv = blob.view(NPFP8)
        bv[:, :, OFF_COEF:OFF_XA] = cc
        bv[:, :, OFF_XA:OFF_NZA] = xc[:, :, :2].reshape(B_PER, 128, 2048)
        bv[:, :, OFF_NZA:OFF_XB] = nc_[:, :, :4].reshape(B_PER, 128, 1024)
        bv[:, :, OFF_XB:OFF_NZB] = xc[:, :, 2:].reshape(B_PER, 128, 2048)
        bv[:, :, OFF_NZB:OFF_BIAS] = nc_[:, :, 4:].reshape(B_PER, 128, 1024)
        # per-batch bias column as 4 raw fp32 bytes per partition
        bcol = bias[:, s].T.reshape(B_PER, 128, 1)  # [B_PER, 128, 1] fp32
        blob[:, :, OFF_BIAS:BLOB] = np.ascontiguousarray(bcol).view(np.uint8)
        in_maps.append({"data": blob})
    return in_maps


def kernel(x_0, noise, W, b, temb, t, **_ignored):
    nc = _get_nc()
    in_maps = make_in_maps(x_0, noise, W, b, temb, t)
    res = run_bass_kernel_spmd(nc, in_maps, list(range(N_CORES)))
    total = 0.0
    for c in range(N_CORES):
        total += float(res.results[c]["out"].astype(np.float64).sum())
    return np.float32(total)
